# Initial kernel scaffold
#
"""CfC RNN kernel for Trainium2 (8 NeuronCores, batch-data-parallel).

Model (per step, reference semantics, ts = 1.0):
    z_in = concat([x_t, h])                      # [B, I+H] = [B, 768]
    z1 = 1.7159*tanh(0.666*(z_in @ wb1.T + bb1)) # [B, 1024]
    z2 = 1.7159*tanh(0.666*(z1 @ wb2.T + bb2))   # [B, 1024]
    ff1 = tanh(z2 @ wff1.T + bff1)               # [B, 512]
    ff2 = tanh(z2 @ wff2.T + bff2)
    t   = sigmoid(z2 @ (wta+wtb).T + (bta+btb))  # ta*1+tb folded
    h'  = ff1 + t*(ff2-ff1)

Device layout is dim-major everywhere: [dim -> 128 partitions, batch -> free].
Algebraic folds (host-side):
  - store z1' = tanh(0.666*pre1)  (the 1.7159 is folded into wb2)
  - store z2' = tanh(0.666*pre2)  (the 1.7159 is folded into the head weights)
  - t_a*ts + t_b with ts=1 == one matmul with (wta+wtb), bias (bta+btb)
"""

import sys

sys.path.insert(0, "/opt/trn_rl_repo")

import numpy as np

import concourse.bass as bass
import concourse.tile as tile
from concourse import bacc, mybir
from concourse import bass_utils
from concourse.bass import ds, ts

B, T, I, H, BU = 64, 512, 256, 512, 1024
NCORES = 8
BC = B // NCORES  # batch rows per core
KX = I // 128     # 2  x K-chunks
KH = H // 128     # 4  h K-chunks
M1 = BU // 128    # 8  mm1 out tiles
K2 = BU // 128    # 8  mm2 K-chunks
M2 = BU // 128    # 8  mm2 out tiles
MH = H // 128     # 4  head out tiles

AF = mybir.ActivationFunctionType

# --- build configuration ---------------------------------------------------
CFG = dict(
    dtype_w="bfloat16",  # weights dtype (stationary operand)
    dtype_a="bfloat16",  # activations/x/h dtype (moving operand)
    unroll=2,           # steps per For_i iteration
    hints=False,        # hint_engines on the loop back-edge
    staggered=False,    # staggered_reset loop semaphore recycling
    # timing-knockout flags (break correctness; for diagnosis only)
    ko_dma=False,       # drop the per-step ys DMA
    ko_dyn=False,       # static x index instead of ds(t)
    ko_act=False,       # skip ACT + DVE (PE only)
    ko_mm=False,        # skip matmuls (ACT/DVE only)
    ko_all=False,       # nearly-empty loop body
    onetable=True,      # express sigmoid via tanh => single ACT table set
    outer=1,            # whole-kernel repetitions (timing amplifier)
    wide=True,          # single wide ACT/DVE per phase (requires zero biases)
    preu=True,          # device-precompute u = 0.666*(x @ w1x.T); bf16 only
    wsplit=False,       # hi/lo split weights: W = bf16(W) + bf16(W - bf16(W))
    asplit=False,       # hi/lo split activations (z1/z2/h/x); wide mode only
    ldwopt=False,       # pass --enable-ldw-opt=true to walrus (dedups LDWEIGHTS)
    abufs=2,            # acts tile-pool buffers
    pbufs=2,            # psum tile-pool buffers
)


_LDWOPT_PATCHED = False


def _patch_ldwopt():
    global _LDWOPT_PATCHED
    if _LDWOPT_PATCHED:
        return
    _LDWOPT_PATCHED = True
    orig = bass_utils.run_command

    def patched(cmd, *a, **kw):
        if isinstance(cmd, list):
            cmd = ["--enable-ldw-opt=true" if c == "--enable-ldw-opt=false" else c
                   for c in cmd]
        return orig(cmd, *a, **kw)

    bass_utils.run_command = patched


def _dt(name):
    return {"float32": mybir.dt.float32, "bfloat16": mybir.dt.bfloat16,
            "float32r": mybir.dt.float32r}[name]


def build(T_steps=T, cfg=CFG):
    DTW = _dt(cfg["dtype_w"])
    DT = _dt(cfg["dtype_a"])
    nc = bacc.Bacc("TRN2", target_bir_lowering=False, debug=False,
                   num_devices=NCORES)

    f32 = mybir.dt.float32
    NA = 2 if cfg["asplit"] else 1
    xT_d = nc.dram_tensor("xT", [128, T, KX, NA * BC], DT, kind="ExternalInput").ap()
    NW = 2 if cfg["wsplit"] else 1
    w1_d = nc.dram_tensor("w1", [128, NW, KX + KH, BU], DTW, kind="ExternalInput").ap()
    w2_d = nc.dram_tensor("w2", [128, NW, K2, BU], DTW, kind="ExternalInput").ap()
    wf1_d = nc.dram_tensor("wf1", [128, NW, K2, H], DTW, kind="ExternalInput").ap()
    wf2_d = nc.dram_tensor("wf2", [128, NW, K2, H], DTW, kind="ExternalInput").ap()
    wt_d = nc.dram_tensor("wt", [128, NW, K2, H], DTW, kind="ExternalInput").ap()
    bias_d = nc.dram_tensor("biases", [128, 28], f32, kind="ExternalInput").ap()
    ys_d = nc.dram_tensor("ys", [T, 128, KH * BC], f32, kind="ExternalOutput").ap()

    with tile.TileContext(nc) as tc:
        with tc.tile_pool(name="weights", bufs=1) as wp, \
             tc.tile_pool(name="state", bufs=1) as sp, \
             tc.tile_pool(name="acts", bufs=cfg["abufs"]) as ap_, \
             tc.tile_pool(name="psum", bufs=cfg["pbufs"], space="PSUM") as pp:
            xT = wp.tile([128, T, KX, NA * BC], DT)
            w1 = wp.tile([128, NW, KX + KH, BU], DTW)
            w2 = wp.tile([128, NW, K2, BU], DTW)
            wf1 = wp.tile([128, NW, K2, H], DTW)
            wf2 = wp.tile([128, NW, K2, H], DTW)
            wt = wp.tile([128, NW, K2, H], DTW)
            bia = wp.tile([128, 28], f32)
            for sb_t, dr in ((xT, xT_d), (w1, w1_d), (w2, w2_d), (wf1, wf1_d),
                             (wf2, wf2_d), (wt, wt_d), (bia, bias_d)):
                nc.sync.dma_start(sb_t[:], dr[:])

            h = sp.tile([128, KH, NA * BC], DT)  # recurrent state, dim-major
            h32 = sp.tile([128, KH * BC], f32)   # fp32 copy for output DMA
            if DT == f32 and not cfg["asplit"]:
                h = h32.rearrange("p (c b) -> p c b", c=KH)

            u_sb = None
            if cfg["preu"]:
                u_sb = wp.tile([128, T_steps, M1 * BC], DT)
                TB = 512 // BC                   # time-block per psum bank
                for m in range(M1):
                    for nb in range((T_steps + TB - 1) // TB):
                        nsz = min(TB, T_steps - nb * TB)
                        pu = pp.tile([128, TB * BC], f32, tag="pu")
                        for p in range(NW):
                            for k in range(KX):
                                for a in range(NA):
                                    nc.tensor.matmul(
                                        pu[:, 0:nsz * BC],
                                        w1[:, p, k, ts(m, 128)],
                                        xT[:, nb * TB:nb * TB + nsz, k, ts(a, BC)],
                                        start=(p == 0 and k == 0 and a == 0),
                                        stop=(p == NW - 1 and k == KX - 1
                                              and a == NA - 1))
                        nc.scalar.activation(
                            u_sb[:, nb * TB:nb * TB + nsz, ts(m, BC)],
                            pu[:, 0:nsz * BC], AF.Copy, bias=0.0, scale=0.666)

            def step(t_idx):
                if cfg["ko_all"]:
                    d0 = ap_.tile([128, BC], f32, tag="d0")
                    nc.vector.memset(d0[:], 0.0)
                    return
                x_idx = ds(t_idx, 1) if not cfg["ko_dyn"] else ds(0, 1)
                if not cfg["ko_act"]:
                    z1 = ap_.tile([128, M1, NA * BC], DT, tag="z1")
                    z2 = ap_.tile([128, M2, NA * BC], DT, tag="z2")
                    if cfg["asplit"]:
                        z1f = ap_.tile([128, M1 * BC], f32, tag="z1f")
                        z2f = ap_.tile([128, M2 * BC], f32, tag="z2f")
                    ff1 = ap_.tile([128, MH * BC], f32, tag="ff1")
                    ff2 = ap_.tile([128, MH * BC], f32, tag="ff2")
                    tt = ap_.tile([128, MH * BC], f32, tag="tt")
                else:
                    z1 = z2 = h
                    ff1 = ff2 = tt = h
                if not cfg["ko_mm"]:
                    z1ps = pp.tile([128, M1 * BC], f32, tag="z1ps")
                    z2ps = pp.tile([128, M2 * BC], f32, tag="z2ps")
                    hps = pp.tile([128, 3 * MH * BC], f32, tag="hps")

                # ---- mm1: z1pre = [x_t; h] @ wb1.T  (K = 2 x-chunks + 4 h-chunks)
                for m in range(M1 if not cfg["ko_mm"] else 0):
                    first = True
                    for p in range(NW):
                        if not cfg["preu"]:
                            for k in range(KX):
                                for a in range(NA):
                                    nc.tensor.matmul(
                                        z1ps[:, ts(m, BC)],
                                        w1[:, p, k, ts(m, 128)],
                                        xT[:, x_idx, k, ts(a, BC)],
                                        start=first, stop=False)
                                    first = False
                        for k in range(KH):
                            for a in range(NA):
                                nc.tensor.matmul(
                                    z1ps[:, ts(m, BC)],
                                    w1[:, p, KX + k, ts(m, 128)],
                                    h[:, k, ts(a, BC)],
                                    start=first,
                                    stop=(p == NW - 1 and k == KH - 1
                                          and a == NA - 1))
                                first = False
                # z1 = tanh(0.666*pre + 0.666*bb1)
                def split_phase(dst, dstf, src_ps, scale):
                    # dst: [128, M, 2*BC] hi/lo bf16; dstf: [128, M*BC] f32
                    M = dst.shape[1]
                    dst3 = dst.rearrange("p m (a b) -> p m a b", a=NA)
                    nc.scalar.activation(dstf[:], src_ps[:], AF.Tanh, scale=scale)
                    dfv = dstf.rearrange("p (m b) -> p m b", m=M)
                    nc.scalar.activation(dst3[:, :, 0, :], dfv[:], AF.Copy)
                    nc.vector.tensor_sub(dst3[:, :, 1, :], dfv[:], dst3[:, :, 0, :])

                if cfg["preu"] and not cfg["ko_act"]:
                    zpre = ap_.tile([128, M1 * BC], f32, tag="zpre")
                    nc.vector.scalar_tensor_tensor(
                        zpre[:], z1ps[:], 0.666, u_sb[:, ds(t_idx, 1), :],
                        mybir.AluOpType.mult, mybir.AluOpType.add)
                    if cfg["asplit"]:
                        split_phase(z1, z1f, zpre, 1.0)
                    else:
                        nc.scalar.activation(z1.rearrange("p m b -> p (m b)"),
                                             zpre[:], AF.Tanh)
                elif cfg["wide"] and not cfg["ko_act"]:
                    if cfg["asplit"]:
                        split_phase(z1, z1f, z1ps, 0.666)
                    else:
                        nc.scalar.activation(z1.rearrange("p m b -> p (m b)"),
                                             z1ps[:], AF.Tanh, scale=0.666)
                else:
                    for m in range(M1 if not cfg["ko_act"] else 0):
                        nc.scalar.activation(z1[:, m, :], z1ps[:, ts(m, BC)],
                                             AF.Tanh, bias=bia[:, m:m + 1], scale=0.666)

                # ---- mm2: z2pre = z1 @ (1.7159*wb2).T
                for m in range(M2 if not cfg["ko_mm"] else 0):
                    for p in range(NW):
                        for k in range(K2):
                            for a in range(NA):
                                nc.tensor.matmul(
                                    z2ps[:, ts(m, BC)],
                                    w2[:, p, k, ts(m, 128)],
                                    z1[:, k if not cfg["ko_act"] else k % KH,
                                       ts(a, BC)],
                                    start=(p == 0 and k == 0 and a == 0),
                                    stop=(p == NW - 1 and k == K2 - 1
                                          and a == NA - 1))
                if cfg["wide"] and not cfg["ko_act"]:
                    if cfg["asplit"]:
                        split_phase(z2, z2f, z2ps, 0.666)
                    else:
                        nc.scalar.activation(z2.rearrange("p m b -> p (m b)"),
                                             z2ps[:], AF.Tanh, scale=0.666)
                else:
                    for m in range(M2 if not cfg["ko_act"] else 0):
                        nc.scalar.activation(z2[:, m, :], z2ps[:, ts(m, BC)],
                                             AF.Tanh, bias=bia[:, 8 + m:9 + m], scale=0.666)

                # ---- heads: ff1, ff2, t (weights pre-scaled by 1.7159)
                for hd, w_sb in enumerate((wf1, wf2, wt) if not cfg["ko_mm"] else ()):
                    for m in range(MH):
                        for p in range(NW):
                            for k in range(K2):
                                for a in range(NA):
                                    nc.tensor.matmul(
                                        hps[:, ts(hd * MH + m, BC)],
                                        w_sb[:, p, k, ts(m, 128)],
                                        z2[:, k if not cfg["ko_act"] else k % KH,
                                           ts(a, BC)],
                                        start=(p == 0 and k == 0 and a == 0),
                                        stop=(p == NW - 1 and k == K2 - 1
                                              and a == NA - 1))
                if cfg["wide"] and not cfg["ko_act"]:
                    # ff1|ff2 in one tanh over hps cols [0, 2*MH*BC)
                    nc.scalar.activation(ff1[:], hps[:, 0:MH * BC], AF.Tanh)
                    nc.scalar.activation(ff2[:], hps[:, MH * BC:2 * MH * BC], AF.Tanh)
                    nc.scalar.activation(tt[:], hps[:, 2 * MH * BC:3 * MH * BC],
                                         AF.Tanh, scale=0.5)
                else:
                    for m in range(MH if not cfg["ko_act"] else 0):
                        nc.scalar.activation(ff1[:, ts(m, BC)], hps[:, ts(m, BC)],
                                             AF.Tanh, bias=bia[:, 16 + m:17 + m])
                    for m in range(MH if not cfg["ko_act"] else 0):
                        nc.scalar.activation(ff2[:, ts(m, BC)], hps[:, ts(MH + m, BC)],
                                             AF.Tanh, bias=bia[:, 20 + m:21 + m])
                    for m in range(MH if not cfg["ko_act"] else 0):
                        if cfg["onetable"]:
                            nc.scalar.activation(tt[:, ts(m, BC)], hps[:, ts(2 * MH + m, BC)],
                                                 AF.Tanh, bias=bia[:, 24 + m:25 + m],
                                                 scale=0.5)
                        else:
                            nc.scalar.activation(tt[:, ts(m, BC)], hps[:, ts(2 * MH + m, BC)],
                                                 AF.Sigmoid, bias=bia[:, 24 + m:25 + m])

                # ---- h' = ff1 + t*(ff2-ff1); onetable: t = 0.5*(1+tt_raw)
                if cfg["wide"] and not cfg["ko_act"]:
                    W = MH * BC
                    d = ap_.tile([128, W], f32, tag="d")
                    e = ap_.tile([128, W], f32, tag="e")
                    nc.vector.tensor_sub(d[:], ff2[:], ff1[:])
                    nc.vector.tensor_scalar_add(e[:], tt[:], 1.0)
                    nc.vector.tensor_mul(d[:], d[:], e[:])
                    nc.vector.scalar_tensor_tensor(
                        h32[:], d[:], 0.5, ff1[:],
                        mybir.AluOpType.mult, mybir.AluOpType.add)
                for c in range(0 if (cfg["wide"] or cfg["ko_act"]) else KH):
                    d = ap_.tile([128, BC], f32, tag="d")
                    e = ap_.tile([128, BC], f32, tag="e")
                    nc.vector.tensor_sub(d[:], ff2[:, ts(c, BC)], ff1[:, ts(c, BC)])
                    if cfg["onetable"]:
                        nc.vector.tensor_scalar_add(e[:], tt[:, ts(c, BC)], 1.0)
                        nc.vector.tensor_mul(d[:], d[:], e[:])
                        nc.vector.scalar_tensor_tensor(
                            h32[:, ts(c, BC)], d[:], 0.5, ff1[:, ts(c, BC)],
                            mybir.AluOpType.mult, mybir.AluOpType.add)
                    else:
                        nc.vector.tensor_mul(e[:], d[:], tt[:, ts(c, BC)])
                        nc.vector.tensor_add(h32[:, ts(c, BC)], e[:], ff1[:, ts(c, BC)])
                if not cfg["ko_act"] and (DT != f32 or cfg["asplit"]):
                    h3 = h.rearrange("p c (a b) -> p c a b", a=NA)
                    h32v = h32.rearrange("p (c b) -> p c b", c=KH)
                    nc.scalar.activation(h3[:, :, 0, :], h32v[:], AF.Copy)
                    if cfg["asplit"]:
                        nc.vector.tensor_sub(h3[:, :, 1, :], h32v[:], h3[:, :, 0, :])

                if not cfg["ko_dma"]:
                    nc.sync.dma_start(ys_d[ds(t_idx, 1), :, :], h32[:])

            U = cfg["unroll"]
            hint = ()
            if cfg["hints"]:
                hint = (mybir.EngineType.PE, mybir.EngineType.Activation,
                        mybir.EngineType.DVE)

            def t_loop():
                nc.vector.memset(h[:], 0.0)
                if DT != f32 or cfg["asplit"]:
                    nc.vector.memset(h32[:], 0.0)
                with tc.For_i(0, T_steps, U, hint_engines=hint,
                              staggered_reset=cfg["staggered"]) as i:
                    for u in range(U):
                        step(i + u if u else i)

            if cfg["outer"] == 1:
                t_loop()
            else:
                with tc.For_i(0, cfg["outer"], 1):
                    t_loop()

    nc.compile()
    return nc


# --- host side -------------------------------------------------------------

def _chunk(w2d):
    """[K, M] row-chunked to [128, K//128, M]."""
    K, M = w2d.shape
    return np.ascontiguousarray(
        w2d.reshape(K // 128, 128, M).transpose(1, 0, 2))


def _wprep(w2d, np_dt_w, wsplit):
    """[K, M] -> [128, NW, K//128, M] in np_dt_w (hi/lo split if wsplit)."""
    c = _chunk(w2d.astype(np.float32))
    hi = c.astype(np_dt_w)
    if not wsplit:
        return hi[:, None]
    lo = (c - hi.astype(np.float32)).astype(np_dt_w)
    return np.ascontiguousarray(np.stack([hi, lo], axis=1))


def _prep(np_dt_w, np_dt_a, x, wb1, bb1, wb2, bb2, wff1, bff1, wff2, bff2, wta, bta, wtb, btb,
          cfg=None):
    cfg = cfg or CFG
    f32 = np.float32
    ws = cfg["wsplit"]
    w1 = _wprep(wb1.T, np_dt_w, ws)                       # [128, NW, 6, 1024]
    w2 = _wprep((1.7159 * wb2).T, np_dt_w, ws)
    wf1 = _wprep((1.7159 * wff1).T, np_dt_w, ws)
    wf2 = _wprep((1.7159 * wff2).T, np_dt_w, ws)
    wt = _wprep((1.7159 * (wta + wtb)).T, np_dt_w, ws)
    bias = np.zeros((128, 28), f32)
    bias[:, 0:8] = (0.666 * bb1).reshape(8, 128).T
    bias[:, 8:16] = (0.666 * bb2).reshape(8, 128).T
    bias[:, 16:20] = bff1.reshape(4, 128).T
    bias[:, 20:24] = bff2.reshape(4, 128).T
    bias[:, 24:28] = (0.5 if cfg['onetable'] else 1.0) * (bta + btb).reshape(4, 128).T

    in_maps = []
    for c in range(NCORES):
        xc = x[c * BC:(c + 1) * BC].astype(f32)                     # [BC, T, I]
        xTf = np.ascontiguousarray(
            xc.reshape(BC, T, KX, 128).transpose(3, 1, 2, 0))       # [128,T,KX,BC]
        hi = xTf.astype(np_dt_a)
        if cfg["asplit"]:
            lo = (xTf - hi.astype(f32)).astype(np_dt_a)
            xT = np.ascontiguousarray(
                np.stack([hi, lo], axis=3)).reshape(128, T, KX, 2 * BC)
        else:
            xT = hi
        in_maps.append(dict(xT=xT, w1=w1, w2=w2, wf1=wf1, wf2=wf2, wt=wt,
                            biases=bias))
    return in_maps


_CACHE = {}
LAST_EXEC_NS = None
TRACE = False
TIME_RUNS = 0


def kernel(**inputs):
    global LAST_EXEC_NS
    import ml_dtypes
    inputs = {k: np.asarray(v) for k, v in inputs.items()}
    # wide mode folds biases away; only valid when all biases are zero
    if (CFG["wide"] or CFG["preu"]) and any(
            np.any(np.asarray(inputs[k]) != 0)
            for k in ("bb1", "bb2", "bff1", "bff2", "bta", "btb")):
        CFG["wide"] = False
        CFG["preu"] = False

    def npdt(s):
        return {"float32": np.float32, "bfloat16": ml_dtypes.bfloat16}[s]

    key = tuple(sorted(CFG.items()))
    if key not in _CACHE:
        _CACHE[key] = build(T, CFG)
    nc = _CACHE[key]
    in_maps = _prep(npdt(CFG["dtype_w"]), npdt(CFG["dtype_a"]), cfg=CFG, **inputs)
    res = bass_utils.run_bass_kernel_spmd(nc, in_maps, core_ids=list(range(NCORES)),
                                          trace=TRACE)
    LAST_EXEC_NS = res.exec_time_ns
    if TIME_RUNS:
        import time
        walls = []
        for _ in range(TIME_RUNS):
            t0 = time.time()
            bass_utils.run_bass_kernel_spmd(nc, in_maps,
                                            core_ids=list(range(NCORES)))
            walls.append(time.time() - t0)
        # wall includes host<->device transfer + axon overhead; min is an
        # upper bound on device exec time
        LAST_EXEC_NS = int(min(walls) * 1e9)
        print(f"timed runs (wall s): {[round(w,3) for w in walls]}")
    out = np.empty((B, T, H), np.float32)
    for c in range(NCORES):
        ys = res.results[c]["ys"]                                   # [T, 128, KH*BC]
        out[c * BC:(c + 1) * BC] = (
            ys.reshape(T, 128, KH, BC).transpose(3, 0, 2, 1).reshape(BC, T, H))
    return out



# revision 3
# speedup vs baseline: 51.6981x; 51.6981x over previous
"""CfC RNN kernel for Trainium2 (8 NeuronCores, batch-data-parallel).

Model (per step, reference semantics, ts = 1.0):
    z_in = concat([x_t, h])                      # [B, I+H] = [B, 768]
    z1 = 1.7159*tanh(0.666*(z_in @ wb1.T + bb1)) # [B, 1024]
    z2 = 1.7159*tanh(0.666*(z1 @ wb2.T + bb2))   # [B, 1024]
    ff1 = tanh(z2 @ wff1.T + bff1)               # [B, 512]
    ff2 = tanh(z2 @ wff2.T + bff2)
    t   = sigmoid(z2 @ (wta+wtb).T + (bta+btb))  # ta*1+tb folded
    h'  = ff1 + t*(ff2-ff1)

Device layout is dim-major everywhere: [dim -> 128 partitions, batch -> free].
Algebraic folds (host-side):
  - store z1' = tanh(0.666*pre1)  (the 1.7159 is folded into wb2)
  - store z2' = tanh(0.666*pre2)  (the 1.7159 is folded into the head weights)
  - t_a*ts + t_b with ts=1 == one matmul with (wta+wtb), bias (bta+btb)
"""

import sys

sys.path.insert(0, "/opt/trn_rl_repo")

import numpy as np

import concourse.bass as bass
import concourse.tile as tile
from concourse import bacc, mybir
from concourse import bass_utils
from concourse.bass import ds, ts

B, T, I, H, BU = 64, 512, 256, 512, 1024
NCORES = 8
BC = B // NCORES  # batch rows per core
KX = I // 128     # 2  x K-chunks
KH = H // 128     # 4  h K-chunks
M1 = BU // 128    # 8  mm1 out tiles
K2 = BU // 128    # 8  mm2 K-chunks
M2 = BU // 128    # 8  mm2 out tiles
MH = H // 128     # 4  head out tiles

AF = mybir.ActivationFunctionType

# --- build configuration ---------------------------------------------------
CFG = dict(
    dtype_w="bfloat16",  # weights dtype (stationary operand)
    dtype_a="bfloat16",  # activations/x/h dtype (moving operand)
    unroll=2,           # steps per For_i iteration
    hints=False,        # hint_engines on the loop back-edge
    staggered=False,    # staggered_reset loop semaphore recycling
    # timing-knockout flags (break correctness; for diagnosis only)
    ko_dma=False,       # drop the per-step ys DMA
    ko_dyn=False,       # static x index instead of ds(t)
    ko_act=False,       # skip ACT + DVE (PE only)
    ko_mm=False,        # skip matmuls (ACT/DVE only)
    ko_all=False,       # nearly-empty loop body
    onetable=True,      # express sigmoid via tanh => single ACT table set
    outer=1,            # whole-kernel repetitions (timing amplifier)
    wide=True,          # single wide ACT/DVE per phase (requires zero biases)
    preu=True,          # device-precompute u = 0.666*(x @ w1x.T); bf16 only
    wsplit=False,       # hi/lo split weights: W = bf16(W) + bf16(W - bf16(W))
    asplit=False,       # hi/lo split activations (z1/z2/h/x); wide mode only
    ldwopt=False,       # pass --enable-ldw-opt=true to walrus (dedups LDWEIGHTS)
    abufs=2,            # acts tile-pool buffers
    pbufs=2,            # psum tile-pool buffers
    static=False,       # python-unrolled T loop (for TimelineSim / small T)
)


_LDWOPT_PATCHED = False


def _patch_ldwopt():
    global _LDWOPT_PATCHED
    if _LDWOPT_PATCHED:
        return
    _LDWOPT_PATCHED = True
    orig = bass_utils.run_command

    def patched(cmd, *a, **kw):
        if isinstance(cmd, list):
            cmd = ["--enable-ldw-opt=true" if c == "--enable-ldw-opt=false" else c
                   for c in cmd]
        return orig(cmd, *a, **kw)

    bass_utils.run_command = patched


def _dt(name):
    return {"float32": mybir.dt.float32, "bfloat16": mybir.dt.bfloat16,
            "float32r": mybir.dt.float32r}[name]


def build(T_steps=T, cfg=CFG):
    DTW = _dt(cfg["dtype_w"])
    DT = _dt(cfg["dtype_a"])
    nc = bacc.Bacc("TRN2", target_bir_lowering=False, debug=False,
                   num_devices=NCORES)

    f32 = mybir.dt.float32
    NA = 2 if cfg["asplit"] else 1
    xT_d = nc.dram_tensor("xT", [128, T, KX, NA * BC], DT, kind="ExternalInput").ap()
    NW = 2 if cfg["wsplit"] else 1
    w1_d = nc.dram_tensor("w1", [128, NW, KX + KH, BU], DTW, kind="ExternalInput").ap()
    w2_d = nc.dram_tensor("w2", [128, NW, K2, BU], DTW, kind="ExternalInput").ap()
    wf1_d = nc.dram_tensor("wf1", [128, NW, K2, H], DTW, kind="ExternalInput").ap()
    wf2_d = nc.dram_tensor("wf2", [128, NW, K2, H], DTW, kind="ExternalInput").ap()
    wt_d = nc.dram_tensor("wt", [128, NW, K2, H], DTW, kind="ExternalInput").ap()
    bias_d = nc.dram_tensor("biases", [128, 28], f32, kind="ExternalInput").ap()
    ys_d = nc.dram_tensor("ys", [T, 128, KH * BC], f32, kind="ExternalOutput").ap()

    with tile.TileContext(nc) as tc:
        with tc.tile_pool(name="weights", bufs=1) as wp, \
             tc.tile_pool(name="state", bufs=1) as sp, \
             tc.tile_pool(name="acts", bufs=cfg["abufs"]) as ap_, \
             tc.tile_pool(name="psum", bufs=cfg["pbufs"], space="PSUM") as pp:
            xT = wp.tile([128, T, KX, NA * BC], DT)
            w1 = wp.tile([128, NW, KX + KH, BU], DTW)
            w2 = wp.tile([128, NW, K2, BU], DTW)
            wf1 = wp.tile([128, NW, K2, H], DTW)
            wf2 = wp.tile([128, NW, K2, H], DTW)
            wt = wp.tile([128, NW, K2, H], DTW)
            bia = wp.tile([128, 28], f32)
            for sb_t, dr in ((xT, xT_d), (w1, w1_d), (w2, w2_d), (wf1, wf1_d),
                             (wf2, wf2_d), (wt, wt_d), (bia, bias_d)):
                nc.sync.dma_start(sb_t[:], dr[:])

            h = sp.tile([128, KH, NA * BC], DT)  # recurrent state, dim-major
            h32 = sp.tile([128, KH * BC], f32)   # fp32 copy for output DMA
            if DT == f32 and not cfg["asplit"]:
                h = h32.rearrange("p (c b) -> p c b", c=KH)

            u_sb = None
            if cfg["preu"]:
                u_sb = wp.tile([128, T_steps, M1 * BC], DT)
                TB = 512 // BC                   # time-block per psum bank
                for m in range(M1):
                    for nb in range((T_steps + TB - 1) // TB):
                        nsz = min(TB, T_steps - nb * TB)
                        pu = pp.tile([128, TB * BC], f32, tag="pu")
                        for p in range(NW):
                            for k in range(KX):
                                for a in range(NA):
                                    nc.tensor.matmul(
                                        pu[:, 0:nsz * BC],
                                        w1[:, p, k, ts(m, 128)],
                                        xT[:, nb * TB:nb * TB + nsz, k, ts(a, BC)],
                                        start=(p == 0 and k == 0 and a == 0),
                                        stop=(p == NW - 1 and k == KX - 1
                                              and a == NA - 1))
                        nc.scalar.activation(
                            u_sb[:, nb * TB:nb * TB + nsz, ts(m, BC)],
                            pu[:, 0:nsz * BC], AF.Copy, bias=0.0, scale=0.666)

            def step(t_idx):
                if cfg["ko_all"]:
                    d0 = ap_.tile([128, BC], f32, tag="d0")
                    nc.vector.memset(d0[:], 0.0)
                    return
                x_idx = ds(t_idx, 1) if not cfg["ko_dyn"] else ds(0, 1)
                if not cfg["ko_act"]:
                    z1 = ap_.tile([128, M1, NA * BC], DT, tag="z1")
                    z2 = ap_.tile([128, M2, NA * BC], DT, tag="z2")
                    if cfg["asplit"]:
                        z1f = ap_.tile([128, M1 * BC], f32, tag="z1f")
                        z2f = ap_.tile([128, M2 * BC], f32, tag="z2f")
                    ff1 = ap_.tile([128, MH * BC], f32, tag="ff1")
                    ff2 = ap_.tile([128, MH * BC], f32, tag="ff2")
                    tt = ap_.tile([128, MH * BC], f32, tag="tt")
                else:
                    z1 = z2 = h
                    ff1 = ff2 = tt = h
                if not cfg["ko_mm"]:
                    z1ps = pp.tile([128, M1 * BC], f32, tag="z1ps")
                    z2ps = pp.tile([128, M2 * BC], f32, tag="z2ps")
                    hps = pp.tile([128, 3 * MH * BC], f32, tag="hps")

                # ---- mm1: z1pre = [x_t; h] @ wb1.T  (K = 2 x-chunks + 4 h-chunks)
                for m in range(M1 if not cfg["ko_mm"] else 0):
                    first = True
                    for p in range(NW):
                        if not cfg["preu"]:
                            for k in range(KX):
                                for a in range(NA):
                                    nc.tensor.matmul(
                                        z1ps[:, ts(m, BC)],
                                        w1[:, p, k, ts(m, 128)],
                                        xT[:, x_idx, k, ts(a, BC)],
                                        start=first, stop=False)
                                    first = False
                        for k in range(KH):
                            for a in range(NA):
                                nc.tensor.matmul(
                                    z1ps[:, ts(m, BC)],
                                    w1[:, p, KX + k, ts(m, 128)],
                                    h[:, k, ts(a, BC)],
                                    start=first,
                                    stop=(p == NW - 1 and k == KH - 1
                                          and a == NA - 1))
                                first = False
                # z1 = tanh(0.666*pre + 0.666*bb1)
                def split_phase(dst, dstf, src_ps, scale):
                    # dst: [128, M, 2*BC] hi/lo bf16; dstf: [128, M*BC] f32
                    M = dst.shape[1]
                    dst3 = dst.rearrange("p m (a b) -> p m a b", a=NA)
                    nc.scalar.activation(dstf[:], src_ps[:], AF.Tanh, scale=scale)
                    dfv = dstf.rearrange("p (m b) -> p m b", m=M)
                    nc.scalar.activation(dst3[:, :, 0, :], dfv[:], AF.Copy)
                    nc.vector.tensor_sub(dst3[:, :, 1, :], dfv[:], dst3[:, :, 0, :])

                if cfg["preu"] and not cfg["ko_act"]:
                    zpre = ap_.tile([128, M1 * BC], f32, tag="zpre")
                    nc.vector.scalar_tensor_tensor(
                        zpre[:], z1ps[:], 0.666, u_sb[:, ds(t_idx, 1), :],
                        mybir.AluOpType.mult, mybir.AluOpType.add)
                    if cfg["asplit"]:
                        split_phase(z1, z1f, zpre, 1.0)
                    else:
                        nc.scalar.activation(z1.rearrange("p m b -> p (m b)"),
                                             zpre[:], AF.Tanh)
                elif cfg["wide"] and not cfg["ko_act"]:
                    if cfg["asplit"]:
                        split_phase(z1, z1f, z1ps, 0.666)
                    else:
                        nc.scalar.activation(z1.rearrange("p m b -> p (m b)"),
                                             z1ps[:], AF.Tanh, scale=0.666)
                else:
                    for m in range(M1 if not cfg["ko_act"] else 0):
                        nc.scalar.activation(z1[:, m, :], z1ps[:, ts(m, BC)],
                                             AF.Tanh, bias=bia[:, m:m + 1], scale=0.666)

                # ---- mm2: z2pre = z1 @ (1.7159*wb2).T
                for m in range(M2 if not cfg["ko_mm"] else 0):
                    for p in range(NW):
                        for k in range(K2):
                            for a in range(NA):
                                nc.tensor.matmul(
                                    z2ps[:, ts(m, BC)],
                                    w2[:, p, k, ts(m, 128)],
                                    z1[:, k if not cfg["ko_act"] else k % KH,
                                       ts(a, BC)],
                                    start=(p == 0 and k == 0 and a == 0),
                                    stop=(p == NW - 1 and k == K2 - 1
                                          and a == NA - 1))
                if cfg["wide"] and not cfg["ko_act"]:
                    if cfg["asplit"]:
                        split_phase(z2, z2f, z2ps, 0.666)
                    else:
                        nc.scalar.activation(z2.rearrange("p m b -> p (m b)"),
                                             z2ps[:], AF.Tanh, scale=0.666)
                else:
                    for m in range(M2 if not cfg["ko_act"] else 0):
                        nc.scalar.activation(z2[:, m, :], z2ps[:, ts(m, BC)],
                                             AF.Tanh, bias=bia[:, 8 + m:9 + m], scale=0.666)

                # ---- heads: ff1, ff2, t (weights pre-scaled by 1.7159)
                for hd, w_sb in enumerate((wf1, wf2, wt) if not cfg["ko_mm"] else ()):
                    for m in range(MH):
                        for p in range(NW):
                            for k in range(K2):
                                for a in range(NA):
                                    nc.tensor.matmul(
                                        hps[:, ts(hd * MH + m, BC)],
                                        w_sb[:, p, k, ts(m, 128)],
                                        z2[:, k if not cfg["ko_act"] else k % KH,
                                           ts(a, BC)],
                                        start=(p == 0 and k == 0 and a == 0),
                                        stop=(p == NW - 1 and k == K2 - 1
                                              and a == NA - 1))
                if cfg["wide"] and not cfg["ko_act"]:
                    # ff1|ff2 in one tanh over hps cols [0, 2*MH*BC)
                    nc.scalar.activation(ff1[:], hps[:, 0:MH * BC], AF.Tanh)
                    nc.scalar.activation(ff2[:], hps[:, MH * BC:2 * MH * BC], AF.Tanh)
                    nc.scalar.activation(tt[:], hps[:, 2 * MH * BC:3 * MH * BC],
                                         AF.Tanh, scale=0.5)
                else:
                    for m in range(MH if not cfg["ko_act"] else 0):
                        nc.scalar.activation(ff1[:, ts(m, BC)], hps[:, ts(m, BC)],
                                             AF.Tanh, bias=bia[:, 16 + m:17 + m])
                    for m in range(MH if not cfg["ko_act"] else 0):
                        nc.scalar.activation(ff2[:, ts(m, BC)], hps[:, ts(MH + m, BC)],
                                             AF.Tanh, bias=bia[:, 20 + m:21 + m])
                    for m in range(MH if not cfg["ko_act"] else 0):
                        if cfg["onetable"]:
                            nc.scalar.activation(tt[:, ts(m, BC)], hps[:, ts(2 * MH + m, BC)],
                                                 AF.Tanh, bias=bia[:, 24 + m:25 + m],
                                                 scale=0.5)
                        else:
                            nc.scalar.activation(tt[:, ts(m, BC)], hps[:, ts(2 * MH + m, BC)],
                                                 AF.Sigmoid, bias=bia[:, 24 + m:25 + m])

                # ---- h' = ff1 + t*(ff2-ff1); onetable: t = 0.5*(1+tt_raw)
                if cfg["wide"] and not cfg["ko_act"]:
                    W = MH * BC
                    d = ap_.tile([128, W], f32, tag="d")
                    e = ap_.tile([128, W], f32, tag="e")
                    nc.vector.tensor_sub(d[:], ff2[:], ff1[:])
                    nc.vector.tensor_scalar_add(e[:], tt[:], 1.0)
                    nc.vector.tensor_mul(d[:], d[:], e[:])
                    nc.vector.scalar_tensor_tensor(
                        h32[:], d[:], 0.5, ff1[:],
                        mybir.AluOpType.mult, mybir.AluOpType.add)
                for c in range(0 if (cfg["wide"] or cfg["ko_act"]) else KH):
                    d = ap_.tile([128, BC], f32, tag="d")
                    e = ap_.tile([128, BC], f32, tag="e")
                    nc.vector.tensor_sub(d[:], ff2[:, ts(c, BC)], ff1[:, ts(c, BC)])
                    if cfg["onetable"]:
                        nc.vector.tensor_scalar_add(e[:], tt[:, ts(c, BC)], 1.0)
                        nc.vector.tensor_mul(d[:], d[:], e[:])
                        nc.vector.scalar_tensor_tensor(
                            h32[:, ts(c, BC)], d[:], 0.5, ff1[:, ts(c, BC)],
                            mybir.AluOpType.mult, mybir.AluOpType.add)
                    else:
                        nc.vector.tensor_mul(e[:], d[:], tt[:, ts(c, BC)])
                        nc.vector.tensor_add(h32[:, ts(c, BC)], e[:], ff1[:, ts(c, BC)])
                if not cfg["ko_act"] and (DT != f32 or cfg["asplit"]):
                    h3 = h.rearrange("p c (a b) -> p c a b", a=NA)
                    h32v = h32.rearrange("p (c b) -> p c b", c=KH)
                    nc.scalar.activation(h3[:, :, 0, :], h32v[:], AF.Copy)
                    if cfg["asplit"]:
                        nc.vector.tensor_sub(h3[:, :, 1, :], h32v[:], h3[:, :, 0, :])

                if not cfg["ko_dma"]:
                    nc.sync.dma_start(ys_d[ds(t_idx, 1), :, :], h32[:])

            U = cfg["unroll"]
            hint = ()
            if cfg["hints"]:
                hint = (mybir.EngineType.PE, mybir.EngineType.Activation,
                        mybir.EngineType.DVE)

            def t_loop():
                nc.vector.memset(h[:], 0.0)
                if DT != f32 or cfg["asplit"]:
                    nc.vector.memset(h32[:], 0.0)
                if cfg.get("static"):
                    for i in range(0, T_steps, U):
                        for u in range(U):
                            step(i + u)
                else:
                    with tc.For_i(0, T_steps, U, hint_engines=hint,
                                  staggered_reset=cfg["staggered"]) as i:
                        for u in range(U):
                            step(i + u if u else i)

            if cfg["outer"] == 1:
                t_loop()
            else:
                with tc.For_i(0, cfg["outer"], 1):
                    t_loop()

    nc.compile()
    return nc


# --- host side -------------------------------------------------------------

def _chunk(w2d):
    """[K, M] row-chunked to [128, K//128, M]."""
    K, M = w2d.shape
    return np.ascontiguousarray(
        w2d.reshape(K // 128, 128, M).transpose(1, 0, 2))


def _wprep(w2d, np_dt_w, wsplit):
    """[K, M] -> [128, NW, K//128, M] in np_dt_w (hi/lo split if wsplit)."""
    c = _chunk(w2d.astype(np.float32))
    hi = c.astype(np_dt_w)
    if not wsplit:
        return hi[:, None]
    lo = (c - hi.astype(np.float32)).astype(np_dt_w)
    return np.ascontiguousarray(np.stack([hi, lo], axis=1))


def _prep(np_dt_w, np_dt_a, x, wb1, bb1, wb2, bb2, wff1, bff1, wff2, bff2, wta, bta, wtb, btb,
          cfg=None):
    cfg = cfg or CFG
    f32 = np.float32
    ws = cfg["wsplit"]
    w1 = _wprep(wb1.T, np_dt_w, ws)                       # [128, NW, 6, 1024]
    w2 = _wprep((1.7159 * wb2).T, np_dt_w, ws)
    wf1 = _wprep((1.7159 * wff1).T, np_dt_w, ws)
    wf2 = _wprep((1.7159 * wff2).T, np_dt_w, ws)
    wt = _wprep((1.7159 * (wta + wtb)).T, np_dt_w, ws)
    bias = np.zeros((128, 28), f32)
    bias[:, 0:8] = (0.666 * bb1).reshape(8, 128).T
    bias[:, 8:16] = (0.666 * bb2).reshape(8, 128).T
    bias[:, 16:20] = bff1.reshape(4, 128).T
    bias[:, 20:24] = bff2.reshape(4, 128).T
    bias[:, 24:28] = (0.5 if cfg['onetable'] else 1.0) * (bta + btb).reshape(4, 128).T

    in_maps = []
    for c in range(NCORES):
        xc = x[c * BC:(c + 1) * BC].astype(f32)                     # [BC, T, I]
        xTf = np.ascontiguousarray(
            xc.reshape(BC, T, KX, 128).transpose(3, 1, 2, 0))       # [128,T,KX,BC]
        hi = xTf.astype(np_dt_a)
        if cfg["asplit"]:
            lo = (xTf - hi.astype(f32)).astype(np_dt_a)
            xT = np.ascontiguousarray(
                np.stack([hi, lo], axis=3)).reshape(128, T, KX, 2 * BC)
        else:
            xT = hi
        in_maps.append(dict(xT=xT, w1=w1, w2=w2, wf1=wf1, wf2=wf2, wt=wt,
                            biases=bias))
    return in_maps


_CACHE = {}
LAST_EXEC_NS = None
TRACE = False
TIME_RUNS = 0


def kernel(**inputs):
    global LAST_EXEC_NS
    import ml_dtypes
    inputs = {k: np.asarray(v) for k, v in inputs.items()}
    # wide mode folds biases away; only valid when all biases are zero
    if (CFG["wide"] or CFG["preu"]) and any(
            np.any(np.asarray(inputs[k]) != 0)
            for k in ("bb1", "bb2", "bff1", "bff2", "bta", "btb")):
        CFG["wide"] = False
        CFG["preu"] = False

    def npdt(s):
        return {"float32": np.float32, "bfloat16": ml_dtypes.bfloat16}[s]

    key = tuple(sorted(CFG.items()))
    if key not in _CACHE:
        _CACHE[key] = build(T, CFG)
    nc = _CACHE[key]
    in_maps = _prep(npdt(CFG["dtype_w"]), npdt(CFG["dtype_a"]), cfg=CFG, **inputs)
    res = bass_utils.run_bass_kernel_spmd(nc, in_maps, core_ids=list(range(NCORES)),
                                          trace=TRACE)
    LAST_EXEC_NS = res.exec_time_ns
    if TIME_RUNS:
        import time
        walls = []
        for _ in range(TIME_RUNS):
            t0 = time.time()
            bass_utils.run_bass_kernel_spmd(nc, in_maps,
                                            core_ids=list(range(NCORES)))
            walls.append(time.time() - t0)
        # wall includes host<->device transfer + axon overhead; min is an
        # upper bound on device exec time
        LAST_EXEC_NS = int(min(walls) * 1e9)
        print(f"timed runs (wall s): {[round(w,3) for w in walls]}")
    out = np.empty((B, T, H), np.float32)
    for c in range(NCORES):
        ys = res.results[c]["ys"]                                   # [T, 128, KH*BC]
        out[c * BC:(c + 1) * BC] = (
            ys.reshape(T, 128, KH, BC).transpose(3, 0, 2, 1).reshape(BC, T, H))
    return out

